# revision 13
# baseline (speedup 1.0000x reference)
"""Banded (sliding-window) multi-head attention for TRN2, 8 NeuronCores.

Problem: nn_BaseAttention (B=2, T=4096, C=512, H=8, hd=64, WIN=128).
  qkv = x @ W_qkv ; banded softmax(q k^T / sqrt(hd), |i-j|<=WIN) @ v ; @ W_out + b_out

Sharding: 8 cores = 2 batches x 4 T-chunks of 1024 queries. Each core gets its
x rows plus a 128-row halo on each side (zero-padded at sequence edges) and
full replicated weights; it produces its own [1024, 512] output slice, so the
host-side gather is pure concatenation (no cross-core reduction).

Device pipeline per core (all layouts chosen to avoid transposing activations):
  xT   = x^T via PE transposes                      [C, 1280]
  q^T/k^T = W_qkv-slice^T-free matmuls (lhsT = W)   [hd, rows]  (head-major)
  v    = natural matmuls (lhsT = xT)                [rows, hd]
  S^T  = k^T-stationary matmuls -> PSUM             [keys, qcols]
  +band mask add (shifted-diagonal strip F) ; exp on ACT (edge tiles killed
  via per-partition bias) -> es in SBUF
  O^T  = sum_kt V-stationary matmuls over es        [2*hd, qcols]
  sums = ones-stationary matmuls over es (64-wide replicated rows)
  O^T * recip(sums) -> O_all^T ; Y = O_all^T-stationary @ W_out + b_out
"""

import os
import numpy as np

import concourse.bass as bass
from concourse import bacc
import concourse.mybir as mybir
import concourse.tile as tile
from concourse.bass_utils import run_bass_kernel_spmd
from concourse.masks import make_identity

# ----- problem constants (hardcoded per contest contract) -----
B, T, C = 2, 4096, 512
H, HD, WIN = 8, 64, 128
NCORES = 8
CHUNK = 1024                # queries per core
ROWS = CHUNK + 2 * WIN      # 1280 rows incl. halo
QCW = 512                   # query-chunk width (qcols per S^T tile group)
NQC = CHUNK // QCW          # 2
NKT = (QCW + 2 * WIN) // 128  # 6 key tiles per query-chunk
NEG = -30000.0
SCALE = HD ** -0.5

F32 = mybir.dt.float32
F32R = mybir.dt.float32r
BF16 = mybir.dt.bfloat16
EXP = mybir.ActivationFunctionType.Exp

# per-key-tile geometry: d = key-tile offset rel. to query-chunk start
# true subrect (cols that contain any in-band entry) and the widened matmul
# rect (>=256 wide so float32r matmuls run at full rate).
_KT_GEOM = []
for _kt in range(NKT):
    _d = 128 * _kt - 128
    _tc0 = max(0, _d - 128)
    _tc1 = min(QCW, _d + 256)
    _mc0, _mc1 = _tc0, _tc1
    if _mc1 - _mc0 < 256:  # widen (kt 0 and 5)
        if _mc0 == 0:
            _mc1 = 256
        else:
            _mc0 = QCW - 256
    _ft0 = _tc0 - _d + 128  # column offset into the band strip F
    # accumulating matmuls (O^T / sums): kt 0 opens the PSUM group and must
    # cover the full bank width so the sim's pending-zero state stays uniform
    # for the later partial-width accumulations.
    _ac0, _ac1 = (0, QCW) if _kt == 0 else (_mc0, _mc1)
    _KT_GEOM.append((_tc0, _tc1, _mc0, _mc1, _ft0, _ac0, _ac1))


def build_attention_body(tc, y, xh, wqkv, wout, bout, ebias):
    """Emit the per-core kernel. All APs are DRAM tensors.

    y     [1024, 512] out     xh    [1280, 512] in (halo'd x rows)
    wqkv  [512, 1536]  in (q-block pre-scaled by hd^-0.5 on host)
    wout  [512, 512]   in     bout  [1, 512] in
    ebias [128, 12]    in (col qc*6+kt: 0 or NEG for out-of-sequence key tiles)
    """
    nc = tc.nc
    from contextlib import ExitStack

    with ExitStack() as ctx:
        sb = ctx.enter_context(tc.tile_pool(name="sb", bufs=1))
        pp = ctx.enter_context(tc.tile_pool(name="pp", bufs=1, space="PSUM"))

        # ---- constants / persistent tiles ----
        idn = sb.tile([128, 128], F32, tag="idn", name="idn")
        make_identity(nc, idn[:])
        ones_f = sb.tile([128, 128], F32, tag="ones_f", name="ones_f")
        nc.gpsimd.memset(ones_f[:], 1.0)
        ones = sb.tile([128, 128], F32R, tag="ones", name="ones")
        nc.vector.tensor_copy(ones[:], ones_f[:])
        ones_b = sb.tile([128, 64], BF16, tag="ones_b", name="ones_b")
        nc.vector.tensor_copy(ones_b[:], ones_f[:, 0:64])
        zeros_f = sb.tile([128, 384], F32, tag="zeros_f", name="zeros_f")
        nc.gpsimd.memset(zeros_f[:], 0.0)

        # band strip F[p, t] = 0 if p <= t <= p+256 else NEG
        Fm = sb.tile([128, 384], F32, tag="Fm", name="Fm")
        nc.gpsimd.memset(Fm[:], 0.0)
        nc.gpsimd.affine_select(
            out=Fm[:], in_=Fm[:], compare_op=mybir.AluOpType.is_ge, fill=NEG,
            base=0, pattern=[[1, 384]], channel_multiplier=-1)   # keep t-p >= 0
        nc.gpsimd.affine_select(
            out=Fm[:], in_=Fm[:], compare_op=mybir.AluOpType.is_ge, fill=NEG,
            base=256, pattern=[[-1, 384]], channel_multiplier=1)  # keep p+256-t >= 0

        eb = sb.tile([128, NQC * NKT], F32, tag="eb", name="eb")
        nc.sync.dma_start(eb[:], ebias[:])
        bo = sb.tile([1, C], F32R, tag="bo", name="bo")
        nc.sync.dma_start(bo[:], bout[:].bitcast(F32R))
        wq_sb = []
        for i in range(4):
            w_i = sb.tile([128, 3 * C], F32R, tag=f"wq{i}", name=f"wq{i}")
            nc.sync.dma_start(w_i[:], wqkv[128 * i:128 * (i + 1), :].bitcast(F32R))
            wq_sb.append(w_i)
        wo_sb = []
        for i in range(4):
            w_i = sb.tile([128, C], F32R, tag=f"wo{i}", name=f"wo{i}")
            nc.sync.dma_start(w_i[:], wout[128 * i:128 * (i + 1), :].bitcast(F32R))
            wo_sb.append(w_i)

        xT = [sb.tile([128, ROWS], F32R, tag=f"xT{i}", name=f"xT{i}") for i in range(4)]
        qT = [sb.tile([128, CHUNK], F32R, tag=f"qT{i}", name=f"qT{i}") for i in range(4)]
        kT = [sb.tile([128, ROWS], F32R, tag=f"kT{i}", name=f"kT{i}") for i in range(4)]
        v_sb = [sb.tile([128, C], BF16, tag=f"v{i}", name=f"v{i}") for i in range(10)]

        # static exp-score tiles: es[j][kt][buf]  (j = head within pair)
        ESB = 2
        es = {}
        for j in range(2):
            for kt in range(NKT):
                tc0, tc1, _, _, _, ac0, ac1 = _KT_GEOM[kt]
                for bf in range(ESB):
                    t_e = sb.tile([128, ac1 - ac0], BF16, tag=f"es{j}_{kt}_{bf}",
                                  name=f"es{j}_{kt}_{bf}")
                    es[(j, kt, bf)] = t_e
                    # zero the never-written complement once; those columns
                    # stay 0 forever (exp only ever writes the true subrect).
                    if tc0 - ac0 > 0:
                        nc.vector.tensor_copy(t_e[:, 0:tc0 - ac0],
                                              zeros_f[:, 0:tc0 - ac0])
                    if ac1 - tc1 > 0:
                        nc.vector.tensor_copy(t_e[:, tc1 - ac0:ac1 - ac0],
                                              zeros_f[:, 0:ac1 - tc1])

        # ---- phase A: xT = x^T via PE transposes ----
        for rt in range(10):
            xs = sb.tile([128, C], F32, tag="xs", bufs=3, name="xs")
            nc.sync.dma_start(xs[:], xh[128 * rt:128 * (rt + 1), :])
            for ct in range(4):
                tp = pp.tile([128, 128], F32, tag="gp", bufs=4, name="tp")
                nc.tensor.transpose(tp[:], xs[:, 128 * ct:128 * (ct + 1)], idn[:])
                nc.any.tensor_copy(xT[ct][:, 128 * rt:128 * (rt + 1)], tp[:])

        # ---- phase B: projections ----
        # q^T / k^T: out[feat, rows]; lhsT = W_qkv block, rhs = xT
        for ft in range(8):
            if ft < 4:  # q feats, own rows only (local rows [128, 1152))
                chunks = [(128, 512), (640, 512)]
                dest, doff = qT[ft], -128
            else:       # k feats, all rows
                chunks = [(0, 512), (512, 512), (1024, 256)]
                dest, doff = kT[ft - 4], 0
            for r0, rw in chunks:
                mm = pp.tile([128, QCW], F32, tag="gp", bufs=4, name="mmqk")
                for ct in range(4):
                    nc.tensor.matmul(
                        mm[:, 0:rw],
                        wq_sb[ct][:, 128 * ft:128 * (ft + 1)],
                        xT[ct][:, r0:r0 + rw],
                        start=(ct == 0), stop=(ct == 3))
                nc.any.tensor_copy(dest[:, r0 + doff:r0 + doff + rw], mm[:, 0:rw])
        # v natural: out[rows, vfeat]; lhsT = xT tile, rhs = W_qkv v-block
        for rt in range(10):
            mm = pp.tile([128, QCW], F32, tag="gp", bufs=4, name="mmv")
            for ct in range(4):
                nc.tensor.matmul(
                    mm[:],
                    xT[ct][:, 128 * rt:128 * (rt + 1)],
                    wq_sb[ct][:, 1024:1536],
                    start=(ct == 0), stop=(ct == 3))
            nc.any.tensor_copy(v_sb[rt][:], mm[:])

        # ---- phase C: banded attention ----
        # The sim only supports one PSUM accumulation group per bank region,
        # so each head-in-pair (j) gets its own O^T and sums banks, placed at
        # partition base 64*j so the elementwise normalize stays lane-aligned.
        oall = [[None] * 4 for _ in range(NQC)]
        for qc in range(NQC):
            for pr in range(4):
                otp = [pp.tile([128, QCW], F32, tag=f"av{j}", bufs=1,
                               name=f"otp{j}") for j in range(2)]
                smp = [pp.tile([128, QCW], F32, tag=f"sm{j}", bufs=1,
                               name=f"smp{j}") for j in range(2)]
                esb = (qc * 4 + pr) % ESB
                for kt in range(NKT):
                    tc0, tc1, mc0, mc1, ft0, ac0, ac1 = _KT_GEOM[kt]
                    w = mc1 - mc0
                    l0, l1 = tc0 - ac0, tc1 - ac0  # true region in es cols
                    kcol = 512 * qc + 128 * kt
                    for j in range(2):
                        h = 2 * pr + j
                        p0 = 64 * j
                        sp = pp.tile([128, 384], F32, tag="gp", bufs=4, name="sp")
                        # S^T = k^T (stationary) x q^T (moving)
                        nc.tensor.matmul(
                            sp[:, 0:w],
                            kT[h // 2][p0:p0 + 64, kcol:kcol + 128],
                            qT[h // 2][p0:p0 + 64,
                                             512 * qc + mc0:512 * qc + mc1],
                            start=True, stop=True)
                        # band mask add + exp (edge tiles killed via bias)
                        s0 = tc0 - mc0
                        nc.any.tensor_add(sp[:, s0:s0 + (tc1 - tc0)],
                                          sp[:, s0:s0 + (tc1 - tc0)],
                                          Fm[:, ft0:ft0 + (tc1 - tc0)])
                        e_t = es[(j, kt, esb)]
                        nc.scalar.activation(
                            e_t[:, l0:l1], sp[:, s0:s0 + (tc1 - tc0)], EXP,
                            bias=eb[:, qc * 6 + kt:qc * 6 + kt + 1], scale=1.0)
                        # O^T += V_h (stationary) x es ; sums += ones x es
                        nc.tensor.matmul(
                            otp[j][p0:p0 + 64, ac0:ac1],
                            v_sb[4 * qc + kt][:, 64 * h:64 * h + 64],
                            e_t[:],
                            start=(kt == 0), stop=(kt == NKT - 1))
                        nc.tensor.matmul(
                            smp[j][p0:p0 + 64, ac0:ac1],
                            ones_b[:],
                            e_t[:],
                            start=(kt == 0), stop=(kt == NKT - 1))
                rs = sb.tile([128, QCW], F32, tag="rs", bufs=2, name="rs")
                oa = sb.tile([128, QCW], F32R, tag=f"oa{pr}", bufs=2, name=f"oa{pr}")
                for j in range(2):
                    p0 = 64 * j
                    nc.vector.reciprocal(rs[p0:p0 + 64, :], smp[j][p0:p0 + 64, :])
                    nc.any.tensor_mul(oa[p0:p0 + 64, :], otp[j][p0:p0 + 64, :],
                                      rs[p0:p0 + 64, :])
                oall[qc][pr] = oa

            # ---- phase D: output projection for this query chunk ----
            for rb in range(4):
                yp = pp.tile([128, C], F32, tag="gp", bufs=4, name="yp")
                for pr in range(4):
                    nc.tensor.matmul(
                        yp[:],
                        oall[qc][pr][:, 128 * rb:128 * (rb + 1)],
                        wo_sb[pr][:],
                        start=(pr == 0), stop=False)
                nc.tensor.matmul(yp[:], ones[0:1, 0:128], bo[:],
                                 start=False, stop=True)
                ys = sb.tile([128, C], F32, tag="ys", bufs=3, name="ys")
                nc.any.tensor_copy(ys[:], yp[:])
                r0 = 512 * qc + 128 * rb
                nc.sync.dma_start(y[r0:r0 + 128, :], ys[:])


def build_nc():
    nc = bacc.Bacc("TRN2", target_bir_lowering=False, debug=False,
                   num_devices=NCORES)
    xh = nc.dram_tensor("xh", [ROWS, C], F32, kind="ExternalInput")
    wqkv = nc.dram_tensor("wqkv", [C, 3 * C], F32, kind="ExternalInput")
    wout = nc.dram_tensor("wout", [C, C], F32, kind="ExternalInput")
    bout = nc.dram_tensor("bout", [1, C], F32, kind="ExternalInput")
    ebias = nc.dram_tensor("ebias", [128, NQC * NKT], F32, kind="ExternalInput")
    y = nc.dram_tensor("y", [CHUNK, C], F32, kind="ExternalOutput")
    with tile.TileContext(nc) as tc:
        build_attention_body(tc, y[:], xh[:], wqkv[:], wout[:], bout[:], ebias[:])
    nc.compile()
    return nc


def make_in_maps(x, W_qkv, W_out, b_out):
    """Shard the full inputs into 8 per-core input maps."""
    x = np.asarray(x, dtype=np.float32)
    wqkv = np.asarray(W_qkv, dtype=np.float32).copy()
    wqkv[:, :C] *= SCALE  # fold hd^-0.5 into the q projection
    wout = np.asarray(W_out, dtype=np.float32)
    bo = np.asarray(b_out, dtype=np.float32).reshape(1, C)
    in_maps = []
    for core in range(NCORES):
        b, ch = divmod(core, 4)
        qs = CHUNK * ch
        xhalo = np.zeros((ROWS, C), dtype=np.float32)
        g0, g1 = qs - WIN, qs + CHUNK + WIN
        s0, s1 = max(g0, 0), min(g1, T)
        xhalo[s0 - g0:s1 - g0, :] = x[b, s0:s1, :]
        ebias = np.zeros((128, NQC * NKT), dtype=np.float32)
        for qc in range(NQC):
            for kt in range(NKT):
                k0 = qs + 512 * qc + 128 * kt - WIN  # global key-tile start
                if k0 < 0 or k0 >= T:
                    ebias[:, qc * 6 + kt] = NEG
        in_maps.append(dict(xh=xhalo, wqkv=wqkv, wout=wout, bout=bo, ebias=ebias))
    return in_maps


_CACHED_NC = None


def run_sharded(x, W_qkv, W_out, b_out, **run_kwargs):
    """Build (cached), run on 8 cores, gather. Returns (y_full, BassKernelResults)."""
    global _CACHED_NC
    if _CACHED_NC is None:
        _CACHED_NC = build_nc()
    in_maps = make_in_maps(x, W_qkv, W_out, b_out)
    res = run_bass_kernel_spmd(_CACHED_NC, in_maps, core_ids=list(range(NCORES)),
                               **run_kwargs)
    y_full = np.empty((B, T, C), dtype=np.float32)
    for core in range(NCORES):
        b, ch = divmod(core, 4)
        y_full[b, CHUNK * ch:CHUNK * (ch + 1), :] = res.results[core]["y"]
    return y_full, res


def kernel(x, W_qkv, W_out, b_out):
    y, _ = run_sharded(x, W_qkv, W_out, b_out)
    return y


# revision 18
# speedup vs baseline: 1.1952x; 1.1952x over previous
"""Banded (sliding-window) multi-head attention for TRN2, 8 NeuronCores.

Problem: nn_BaseAttention (B=2, T=4096, C=512, H=8, hd=64, WIN=128).
  qkv = x @ W_qkv ; banded softmax(q k^T / sqrt(hd), |i-j|<=WIN) @ v ; @ W_out + b_out

Sharding: 8 cores = 2 batches x 4 T-chunks of 1024 queries. Each core gets its
x rows plus a 128-row halo on each side (zero-padded at sequence edges) and
full replicated weights; it produces its own [1024, 512] output slice, so the
host-side gather is pure concatenation (no cross-core reduction).

Device pipeline per core (all layouts chosen to avoid transposing activations):
  xT   = x^T via PE transposes                      [C, 1280]
  q^T/k^T = W_qkv-slice^T-free matmuls (lhsT = W)   [hd, rows]  (head-major)
  v    = natural matmuls (lhsT = xT)                [rows, hd]
  S^T  = k^T-stationary matmuls -> PSUM             [keys, qcols]
  +band mask add (shifted-diagonal strip F) ; exp on ACT (edge tiles killed
  via per-partition bias) -> es in SBUF
  O^T  = sum_kt V-stationary matmuls over es        [2*hd, qcols]
  sums = ones-stationary matmuls over es (64-wide replicated rows)
  O^T * recip(sums) -> O_all^T ; Y = O_all^T-stationary @ W_out + b_out
"""

import os
import numpy as np

import concourse.bass as bass
from concourse import bacc
import concourse.mybir as mybir
import concourse.tile as tile
from concourse.bass_utils import run_bass_kernel_spmd
from concourse.masks import make_identity

# ----- problem constants (hardcoded per contest contract) -----
B, T, C = 2, 4096, 512
H, HD, WIN = 8, 64, 128
NCORES = 8
CHUNK = 1024                # queries per core
ROWS = CHUNK + 2 * WIN      # 1280 rows incl. halo
QCW = 512                   # query-chunk width (qcols per S^T tile group)
NQC = CHUNK // QCW          # 2
NKT = (QCW + 2 * WIN) // 128  # 6 key tiles per query-chunk
NEG = -30000.0
SCALE = HD ** -0.5

F32 = mybir.dt.float32
F32R = mybir.dt.float32r
BF16 = mybir.dt.bfloat16
EXP = mybir.ActivationFunctionType.Exp

# per-key-tile geometry: d = key-tile offset rel. to query-chunk start
# true subrect (cols that contain any in-band entry) and the widened matmul
# rect (>=256 wide so float32r matmuls run at full rate).
_KT_GEOM = []
for _kt in range(NKT):
    _d = 128 * _kt - 128
    _tc0 = max(0, _d - 128)
    _tc1 = min(QCW, _d + 256)
    _mc0, _mc1 = _tc0, _tc1
    if _mc1 - _mc0 < 256:  # widen (kt 0 and 5)
        if _mc0 == 0:
            _mc1 = 256
        else:
            _mc0 = QCW - 256
    _ft0 = _tc0 - _d + 128  # column offset into the band strip F
    # accumulating matmuls (O^T / sums): kt 0 opens the PSUM group and must
    # cover the full bank width so the sim's pending-zero state stays uniform
    # for the later partial-width accumulations.
    _ac0, _ac1 = (0, QCW) if _kt == 0 else (_mc0, _mc1)
    _KT_GEOM.append((_tc0, _tc1, _mc0, _mc1, _ft0, _ac0, _ac1))


def build_attention_body(tc, y, xh, wqkv, wout, bout, ebias):
    """Emit the per-core kernel. All APs are DRAM tensors.

    y     [1024, 512] out     xh    [1280, 512] in (halo'd x rows)
    wqkv  [512, 1536]  in (q-block pre-scaled by hd^-0.5 on host)
    wout  [512, 512]   in     bout  [1, 512] in
    ebias [128, 12]    in (col qc*6+kt: 0 or NEG for out-of-sequence key tiles)
    """
    nc = tc.nc
    from contextlib import ExitStack

    with ExitStack() as ctx:
        sb = ctx.enter_context(tc.tile_pool(name="sb", bufs=1))
        pp = ctx.enter_context(tc.tile_pool(name="pp", bufs=1, space="PSUM"))

        # ---- constants / persistent tiles ----
        idn = sb.tile([128, 128], F32, tag="idn", name="idn")
        make_identity(nc, idn[:])
        ones_f = sb.tile([128, 128], F32, tag="ones_f", name="ones_f")
        nc.gpsimd.memset(ones_f[:], 1.0)
        ones = sb.tile([128, 128], F32R, tag="ones", name="ones")
        nc.vector.tensor_copy(ones[:], ones_f[:])
        ones_b = sb.tile([128, 128], BF16, tag="ones_b", name="ones_b")
        nc.vector.tensor_copy(ones_b[:], ones_f[:])
        zeros_f = sb.tile([128, 384], F32, tag="zeros_f", name="zeros_f")
        nc.gpsimd.memset(zeros_f[:], 0.0)

        # band strip F[p, t] = 0 if p <= t <= p+256 else NEG
        Fm = sb.tile([128, 384], F32, tag="Fm", name="Fm")
        nc.gpsimd.memset(Fm[:], 0.0)
        nc.gpsimd.affine_select(
            out=Fm[:], in_=Fm[:], compare_op=mybir.AluOpType.is_ge, fill=NEG,
            base=0, pattern=[[1, 384]], channel_multiplier=-1)   # keep t-p >= 0
        nc.gpsimd.affine_select(
            out=Fm[:], in_=Fm[:], compare_op=mybir.AluOpType.is_ge, fill=NEG,
            base=256, pattern=[[-1, 384]], channel_multiplier=1)  # keep p+256-t >= 0

        eb = sb.tile([128, NQC * NKT], F32, tag="eb", name="eb")
        nc.sync.dma_start(eb[:], ebias[:])
        bo = sb.tile([1, C], BF16, tag="bo", name="bo")
        nc.sync.dma_start(bo[:], bout[:])
        wq_sb = []
        for i in range(4):
            w_i = sb.tile([128, 3 * C], F32R, tag=f"wq{i}", name=f"wq{i}")
            nc.sync.dma_start(w_i[:], wqkv[128 * i:128 * (i + 1), :].bitcast(F32R))
            wq_sb.append(w_i)
        wo_sb = []
        for i in range(4):
            w_i = sb.tile([128, C], BF16, tag=f"wo{i}", name=f"wo{i}")
            nc.sync.dma_start(w_i[:], wout[128 * i:128 * (i + 1), :])
            wo_sb.append(w_i)

        xT = [sb.tile([128, ROWS], F32R, tag=f"xT{i}", name=f"xT{i}") for i in range(4)]
        qT = [sb.tile([128, CHUNK], F32R, tag=f"qT{i}", name=f"qT{i}") for i in range(4)]
        kT = [sb.tile([128, ROWS], F32R, tag=f"kT{i}", name=f"kT{i}") for i in range(4)]
        v_sb = [sb.tile([128, C], BF16, tag=f"v{i}", name=f"v{i}") for i in range(10)]

        # static exp-score tiles: es[j][kt][buf]  (j = head within pair)
        ESB = 2
        es = {}
        for j in range(2):
            for kt in range(NKT):
                tc0, tc1, _, _, _, ac0, ac1 = _KT_GEOM[kt]
                for bf in range(ESB):
                    t_e = sb.tile([128, ac1 - ac0], BF16, tag=f"es{j}_{kt}_{bf}",
                                  name=f"es{j}_{kt}_{bf}")
                    es[(j, kt, bf)] = t_e
                    # zero the never-written complement once; those columns
                    # stay 0 forever (exp only ever writes the true subrect).
                    if tc0 - ac0 > 0:
                        nc.vector.tensor_copy(t_e[:, 0:tc0 - ac0],
                                              zeros_f[:, 0:tc0 - ac0])
                    if ac1 - tc1 > 0:
                        nc.vector.tensor_copy(t_e[:, tc1 - ac0:ac1 - ac0],
                                              zeros_f[:, 0:ac1 - tc1])

        # ---- phase A: xT = x^T via PE transposes ----
        for rt in range(10):
            xs = sb.tile([128, C], F32, tag="xs", bufs=3, name="xs")
            nc.sync.dma_start(xs[:], xh[128 * rt:128 * (rt + 1), :])
            for ct in range(4):
                tp = pp.tile([128, 128], F32, tag="gp", bufs=4, name="tp")
                nc.tensor.transpose(tp[:], xs[:, 128 * ct:128 * (ct + 1)], idn[:])
                nc.any.tensor_copy(xT[ct][:, 128 * rt:128 * (rt + 1)], tp[:])

        # ---- phase B: projections ----
        # q^T / k^T: out[feat, rows]; lhsT = W_qkv block, rhs = xT
        for ft in range(8):
            if ft < 4:  # q feats, own rows only (local rows [128, 1152))
                chunks = [(128, 512), (640, 512)]
                dest, doff = qT[ft], -128
            else:       # k feats, all rows
                chunks = [(0, 512), (512, 512), (1024, 256)]
                dest, doff = kT[ft - 4], 0
            for r0, rw in chunks:
                mm = pp.tile([128, QCW], F32, tag="gp", bufs=4, name="mmqk")
                for ct in range(4):
                    nc.tensor.matmul(
                        mm[:, 0:rw],
                        wq_sb[ct][:, 128 * ft:128 * (ft + 1)],
                        xT[ct][:, r0:r0 + rw],
                        start=(ct == 0), stop=(ct == 3))
                nc.any.tensor_copy(dest[:, r0 + doff:r0 + doff + rw], mm[:, 0:rw])
        # v natural: out[rows, vfeat]; lhsT = xT tile, rhs = W_qkv v-block
        for rt in range(10):
            mm = pp.tile([128, QCW], F32, tag="gp", bufs=4, name="mmv")
            for ct in range(4):
                nc.tensor.matmul(
                    mm[:],
                    xT[ct][:, 128 * rt:128 * (rt + 1)],
                    wq_sb[ct][:, 1024:1536],
                    start=(ct == 0), stop=(ct == 3))
            nc.any.tensor_copy(v_sb[rt][:], mm[:])

        # ---- phase C: banded attention ----
        # The sim only supports one PSUM accumulation group per bank region,
        # so each head-in-pair (j) gets its own O^T and sums banks, placed at
        # partition base 64*j so the elementwise normalize stays lane-aligned.
        oall = [[None] * 4 for _ in range(NQC)]
        for qc in range(NQC):
            for pr in range(4):
                otp = [pp.tile([128, QCW], F32, tag=f"av{j}", bufs=1,
                               name=f"otp{j}") for j in range(2)]
                smp = [pp.tile([128, QCW], F32, tag=f"sm{j}", bufs=1,
                               name=f"smp{j}") for j in range(2)]
                esb = (qc * 4 + pr) % ESB
                for kt in range(NKT):
                    tc0, tc1, mc0, mc1, ft0, ac0, ac1 = _KT_GEOM[kt]
                    w = mc1 - mc0
                    l0, l1 = tc0 - ac0, tc1 - ac0  # true region in es cols
                    kcol = 512 * qc + 128 * kt
                    for j in range(2):
                        h = 2 * pr + j
                        p0 = 64 * j
                        sp = pp.tile([128, 384], F32, tag="gp", bufs=4, name="sp")
                        # S^T = k^T (stationary) x q^T (moving)
                        nc.tensor.matmul(
                            sp[:, 0:w],
                            kT[h // 2][p0:p0 + 64, kcol:kcol + 128],
                            qT[h // 2][p0:p0 + 64,
                                             512 * qc + mc0:512 * qc + mc1],
                            start=True, stop=True)
                        # band mask add + exp (edge tiles killed via bias)
                        s0 = tc0 - mc0
                        nc.any.tensor_add(sp[:, s0:s0 + (tc1 - tc0)],
                                          sp[:, s0:s0 + (tc1 - tc0)],
                                          Fm[:, ft0:ft0 + (tc1 - tc0)])
                        e_t = es[(j, kt, esb)]
                        nc.scalar.activation(
                            e_t[:, l0:l1], sp[:, s0:s0 + (tc1 - tc0)], EXP,
                            bias=eb[:, qc * 6 + kt:qc * 6 + kt + 1], scale=1.0)
                        # O^T += V_h (stationary) x es ; sums += ones x es
                        nc.tensor.matmul(
                            otp[j][p0:p0 + 64, ac0:ac1],
                            v_sb[4 * qc + kt][:, 64 * h:64 * h + 64],
                            e_t[:],
                            start=(kt == 0), stop=(kt == NKT - 1))
                        nc.tensor.matmul(
                            smp[j][p0:p0 + 64, ac0:ac1],
                            ones_b[:, 0:64],
                            e_t[:],
                            start=(kt == 0), stop=(kt == NKT - 1))
                oa = sb.tile([128, QCW], BF16, tag=f"oa{pr}", bufs=2, name=f"oa{pr}")
                ss = sb.tile([128, QCW], F32, tag="ss", bufs=2, name="ss")
                rs = sb.tile([128, QCW], F32, tag="rs", bufs=2, name="rs")
                for j in range(2):
                    p0 = 64 * j
                    nc.any.tensor_copy(ss[p0:p0 + 64, :], smp[j][p0:p0 + 64, :])
                nc.vector.reciprocal(rs[:], ss[:])
                for j in range(2):
                    p0 = 64 * j
                    nc.any.tensor_mul(oa[p0:p0 + 64, :], otp[j][p0:p0 + 64, :],
                                      rs[p0:p0 + 64, :])
                oall[qc][pr] = oa

            # ---- phase D: output projection for this query chunk ----
            for rb in range(4):
                yp = pp.tile([128, C], F32, tag="gp", bufs=4, name="yp")
                for pr in range(4):
                    nc.tensor.matmul(
                        yp[:],
                        oall[qc][pr][:, 128 * rb:128 * (rb + 1)],
                        wo_sb[pr][:],
                        start=(pr == 0), stop=False)
                nc.tensor.matmul(yp[:], ones_b[0:1, :], bo[:],
                                 start=False, stop=True)
                ys = sb.tile([128, C], F32, tag="ys", bufs=3, name="ys")
                nc.any.tensor_copy(ys[:], yp[:])
                r0 = 512 * qc + 128 * rb
                nc.sync.dma_start(y[r0:r0 + 128, :], ys[:])


def build_nc():
    nc = bacc.Bacc("TRN2", target_bir_lowering=False, debug=False,
                   num_devices=NCORES)
    xh = nc.dram_tensor("xh", [ROWS, C], F32, kind="ExternalInput")
    wqkv = nc.dram_tensor("wqkv", [C, 3 * C], F32, kind="ExternalInput")
    wout = nc.dram_tensor("wout", [C, C], BF16, kind="ExternalInput")
    bout = nc.dram_tensor("bout", [1, C], BF16, kind="ExternalInput")
    ebias = nc.dram_tensor("ebias", [128, NQC * NKT], F32, kind="ExternalInput")
    y = nc.dram_tensor("y", [CHUNK, C], F32, kind="ExternalOutput")
    with tile.TileContext(nc) as tc:
        build_attention_body(tc, y[:], xh[:], wqkv[:], wout[:], bout[:], ebias[:])
    nc.compile()
    return nc


def make_in_maps(x, W_qkv, W_out, b_out):
    """Shard the full inputs into 8 per-core input maps."""
    x = np.asarray(x, dtype=np.float32)
    wqkv = np.asarray(W_qkv, dtype=np.float32).copy()
    wqkv[:, :C] *= SCALE  # fold hd^-0.5 into the q projection
    import ml_dtypes
    wout = np.asarray(W_out, dtype=np.float32).astype(ml_dtypes.bfloat16)
    bo = np.asarray(b_out, dtype=np.float32).astype(ml_dtypes.bfloat16).reshape(1, C)
    in_maps = []
    for core in range(NCORES):
        b, ch = divmod(core, 4)
        qs = CHUNK * ch
        xhalo = np.zeros((ROWS, C), dtype=np.float32)
        g0, g1 = qs - WIN, qs + CHUNK + WIN
        s0, s1 = max(g0, 0), min(g1, T)
        xhalo[s0 - g0:s1 - g0, :] = x[b, s0:s1, :]
        ebias = np.zeros((128, NQC * NKT), dtype=np.float32)
        for qc in range(NQC):
            for kt in range(NKT):
                k0 = qs + 512 * qc + 128 * kt - WIN  # global key-tile start
                if k0 < 0 or k0 >= T:
                    ebias[:, qc * 6 + kt] = NEG
        in_maps.append(dict(xh=xhalo, wqkv=wqkv, wout=wout, bout=bo, ebias=ebias))
    return in_maps


_CACHED_NC = None


def run_sharded(x, W_qkv, W_out, b_out, **run_kwargs):
    """Build (cached), run on 8 cores, gather. Returns (y_full, BassKernelResults)."""
    global _CACHED_NC
    if _CACHED_NC is None:
        _CACHED_NC = build_nc()
    in_maps = make_in_maps(x, W_qkv, W_out, b_out)
    res = run_bass_kernel_spmd(_CACHED_NC, in_maps, core_ids=list(range(NCORES)),
                               **run_kwargs)
    y_full = np.empty((B, T, C), dtype=np.float32)
    for core in range(NCORES):
        b, ch = divmod(core, 4)
        y_full[b, CHUNK * ch:CHUNK * (ch + 1), :] = res.results[core]["y"]
    return y_full, res


def kernel(x, W_qkv, W_out, b_out):
    y, _ = run_sharded(x, W_qkv, W_out, b_out)
    return y


# revision 19
# speedup vs baseline: 1.3389x; 1.1203x over previous
"""Banded (sliding-window) multi-head attention for TRN2, 8 NeuronCores.

Problem: nn_BaseAttention (B=2, T=4096, C=512, H=8, hd=64, WIN=128).
  qkv = x @ W_qkv ; banded softmax(q k^T / sqrt(hd), |i-j|<=WIN) @ v ; @ W_out + b_out

Sharding: 8 cores = 2 batches x 4 T-chunks of 1024 queries. Each core gets its
x rows plus a 128-row halo on each side (zero-padded at sequence edges) and
full replicated weights; it produces its own [1024, 512] output slice, so the
host-side gather is pure concatenation (no cross-core reduction).

Device pipeline per core (all layouts chosen to avoid transposing activations):
  xT   = x^T via PE transposes                      [C, 1280]
  q^T/k^T = W_qkv-slice^T-free matmuls (lhsT = W)   [hd, rows]  (head-major)
  v    = natural matmuls (lhsT = xT)                [rows, hd]
  S^T  = k^T-stationary matmuls -> PSUM             [keys, qcols]
  +band mask add (shifted-diagonal strip F) ; exp on ACT (edge tiles killed
  via per-partition bias) -> es in SBUF
  O^T  = sum_kt V-stationary matmuls over es        [2*hd, qcols]
  sums = ones-stationary matmuls over es (64-wide replicated rows)
  O^T * recip(sums) -> O_all^T ; Y = O_all^T-stationary @ W_out + b_out
"""

import os
import numpy as np

import concourse.bass as bass
from concourse import bacc
import concourse.mybir as mybir
import concourse.tile as tile
from concourse.bass_utils import run_bass_kernel_spmd
from concourse.masks import make_identity

# ----- problem constants (hardcoded per contest contract) -----
B, T, C = 2, 4096, 512
H, HD, WIN = 8, 64, 128
NCORES = 8
CHUNK = 1024                # queries per core
ROWS = CHUNK + 2 * WIN      # 1280 rows incl. halo
QCW = 512                   # query-chunk width (qcols per S^T tile group)
NQC = CHUNK // QCW          # 2
NKT = (QCW + 2 * WIN) // 128  # 6 key tiles per query-chunk
NEG = -30000.0
SCALE = HD ** -0.5

F32 = mybir.dt.float32
F32R = mybir.dt.float32r
BF16 = mybir.dt.bfloat16
F16 = mybir.dt.float16
EXP = mybir.ActivationFunctionType.Exp

# per-key-tile geometry: d = key-tile offset rel. to query-chunk start
# true subrect (cols that contain any in-band entry) and the widened matmul
# rect (>=256 wide so float32r matmuls run at full rate).
_KT_GEOM = []
for _kt in range(NKT):
    _d = 128 * _kt - 128
    _tc0 = max(0, _d - 128)
    _tc1 = min(QCW, _d + 256)
    _mc0, _mc1 = _tc0, _tc1
    if _mc1 - _mc0 < 256:  # widen (kt 0 and 5)
        if _mc0 == 0:
            _mc1 = 256
        else:
            _mc0 = QCW - 256
    _ft0 = _tc0 - _d + 128  # column offset into the band strip F
    # accumulating matmuls (O^T / sums): kt 0 opens the PSUM group and must
    # cover the full bank width so the sim's pending-zero state stays uniform
    # for the later partial-width accumulations.
    _ac0, _ac1 = (0, QCW) if _kt == 0 else (_mc0, _mc1)
    _KT_GEOM.append((_tc0, _tc1, _mc0, _mc1, _ft0, _ac0, _ac1))


def build_attention_body(tc, y, xh, wqkv, wout, bout, ebias):
    """Emit the per-core kernel. All APs are DRAM tensors.

    y     [1024, 512] out     xh    [1280, 512] in (halo'd x rows)
    wqkv  [512, 1536]  in (q-block pre-scaled by hd^-0.5 on host)
    wout  [512, 512]   in     bout  [1, 512] in
    ebias [128, 12]    in (col qc*6+kt: 0 or NEG for out-of-sequence key tiles)
    """
    nc = tc.nc
    from contextlib import ExitStack

    with ExitStack() as ctx:
        sb = ctx.enter_context(tc.tile_pool(name="sb", bufs=1))
        pp = ctx.enter_context(tc.tile_pool(name="pp", bufs=1, space="PSUM"))

        # ---- constants / persistent tiles ----
        idn = sb.tile([128, 128], F16, tag="idn", name="idn")
        make_identity(nc, idn[:])
        ones_f = sb.tile([128, 128], F32, tag="ones_f", name="ones_f")
        nc.gpsimd.memset(ones_f[:], 1.0)
        ones_b = sb.tile([128, 128], F16, tag="ones_b", name="ones_b")
        nc.vector.tensor_copy(ones_b[:], ones_f[:])
        zeros_f = sb.tile([128, 384], F32, tag="zeros_f", name="zeros_f")
        nc.gpsimd.memset(zeros_f[:], 0.0)

        # band strip F[p, t] = 0 if p <= t <= p+256 else NEG
        Fm = sb.tile([128, 384], F32, tag="Fm", name="Fm")
        nc.gpsimd.memset(Fm[:], 0.0)
        nc.gpsimd.affine_select(
            out=Fm[:], in_=Fm[:], compare_op=mybir.AluOpType.is_ge, fill=NEG,
            base=0, pattern=[[1, 384]], channel_multiplier=-1)   # keep t-p >= 0
        nc.gpsimd.affine_select(
            out=Fm[:], in_=Fm[:], compare_op=mybir.AluOpType.is_ge, fill=NEG,
            base=256, pattern=[[-1, 384]], channel_multiplier=1)  # keep p+256-t >= 0

        eb = sb.tile([128, NQC * NKT], F32, tag="eb", name="eb")
        nc.sync.dma_start(eb[:], ebias[:])
        bo = sb.tile([1, C], F16, tag="bo", name="bo")
        nc.sync.dma_start(bo[:], bout[:])
        wq_sb = []
        for i in range(4):
            w_i = sb.tile([128, 3 * C], F16, tag=f"wq{i}", name=f"wq{i}")
            nc.sync.dma_start(w_i[:], wqkv[128 * i:128 * (i + 1), :])
            wq_sb.append(w_i)
        wo_sb = []
        for i in range(4):
            w_i = sb.tile([128, C], F16, tag=f"wo{i}", name=f"wo{i}")
            nc.sync.dma_start(w_i[:], wout[128 * i:128 * (i + 1), :])
            wo_sb.append(w_i)

        xT = [sb.tile([128, ROWS], F16, tag=f"xT{i}", name=f"xT{i}") for i in range(4)]
        qT = [sb.tile([128, CHUNK], F16, tag=f"qT{i}", name=f"qT{i}") for i in range(4)]
        kT = [sb.tile([128, ROWS], F16, tag=f"kT{i}", name=f"kT{i}") for i in range(4)]
        v_sb = [sb.tile([128, C], F16, tag=f"v{i}", name=f"v{i}") for i in range(10)]

        # static exp-score tiles: es[j][kt][buf]  (j = head within pair)
        ESB = 2
        es = {}
        for j in range(2):
            for kt in range(NKT):
                tc0, tc1, _, _, _, ac0, ac1 = _KT_GEOM[kt]
                for bf in range(ESB):
                    t_e = sb.tile([128, ac1 - ac0], F16, tag=f"es{j}_{kt}_{bf}",
                                  name=f"es{j}_{kt}_{bf}")
                    es[(j, kt, bf)] = t_e
                    # zero the never-written complement once; those columns
                    # stay 0 forever (exp only ever writes the true subrect).
                    if tc0 - ac0 > 0:
                        nc.vector.tensor_copy(t_e[:, 0:tc0 - ac0],
                                              zeros_f[:, 0:tc0 - ac0])
                    if ac1 - tc1 > 0:
                        nc.vector.tensor_copy(t_e[:, tc1 - ac0:ac1 - ac0],
                                              zeros_f[:, 0:ac1 - tc1])

        # ---- phase A: xT = x^T via PE transposes ----
        for rt in range(10):
            xs = sb.tile([128, C], F16, tag="xs", bufs=3, name="xs")
            nc.sync.dma_start(xs[:], xh[128 * rt:128 * (rt + 1), :])
            for ct in range(4):
                tp = pp.tile([128, 128], F16, tag="gp", bufs=4, name="tp")
                nc.tensor.transpose(tp[:], xs[:, 128 * ct:128 * (ct + 1)], idn[:])
                nc.any.tensor_copy(xT[ct][:, 128 * rt:128 * (rt + 1)], tp[:])

        # ---- phase B: projections ----
        # q^T / k^T: out[feat, rows]; lhsT = W_qkv block, rhs = xT
        for ft in range(8):
            if ft < 4:  # q feats, own rows only (local rows [128, 1152))
                chunks = [(128, 512), (640, 512)]
                dest, doff = qT[ft], -128
            else:       # k feats, all rows
                chunks = [(0, 512), (512, 512), (1024, 256)]
                dest, doff = kT[ft - 4], 0
            for r0, rw in chunks:
                mm = pp.tile([128, QCW], F32, tag="gp", bufs=4, name="mmqk")
                for ct in range(4):
                    nc.tensor.matmul(
                        mm[:, 0:rw],
                        wq_sb[ct][:, 128 * ft:128 * (ft + 1)],
                        xT[ct][:, r0:r0 + rw],
                        start=(ct == 0), stop=(ct == 3))
                nc.any.tensor_copy(dest[:, r0 + doff:r0 + doff + rw], mm[:, 0:rw])
        # v natural: out[rows, vfeat]; lhsT = xT tile, rhs = W_qkv v-block
        for rt in range(10):
            mm = pp.tile([128, QCW], F32, tag="gp", bufs=4, name="mmv")
            for ct in range(4):
                nc.tensor.matmul(
                    mm[:],
                    xT[ct][:, 128 * rt:128 * (rt + 1)],
                    wq_sb[ct][:, 1024:1536],
                    start=(ct == 0), stop=(ct == 3))
            nc.any.tensor_copy(v_sb[rt][:], mm[:])

        # ---- phase C: banded attention ----
        # The sim only supports one PSUM accumulation group per bank region,
        # so each head-in-pair (j) gets its own O^T and sums banks, placed at
        # partition base 64*j so the elementwise normalize stays lane-aligned.
        oall = [[None] * 4 for _ in range(NQC)]
        for qc in range(NQC):
            for pr in range(4):
                otp = [pp.tile([128, QCW], F32, tag=f"av{j}", bufs=1,
                               name=f"otp{j}") for j in range(2)]
                smp = [pp.tile([128, QCW], F32, tag=f"sm{j}", bufs=1,
                               name=f"smp{j}") for j in range(2)]
                esb = (qc * 4 + pr) % ESB
                for kt in range(NKT):
                    tc0, tc1, mc0, mc1, ft0, ac0, ac1 = _KT_GEOM[kt]
                    w = mc1 - mc0
                    l0, l1 = tc0 - ac0, tc1 - ac0  # true region in es cols
                    kcol = 512 * qc + 128 * kt
                    for j in range(2):
                        h = 2 * pr + j
                        p0 = 64 * j
                        sp = pp.tile([128, 384], F32, tag="gp", bufs=4, name="sp")
                        # S^T = k^T (stationary) x q^T (moving)
                        nc.tensor.matmul(
                            sp[:, 0:w],
                            kT[h // 2][p0:p0 + 64, kcol:kcol + 128],
                            qT[h // 2][p0:p0 + 64,
                                             512 * qc + mc0:512 * qc + mc1],
                            start=True, stop=True)
                        # band mask add + exp (edge tiles killed via bias)
                        s0 = tc0 - mc0
                        nc.any.tensor_add(sp[:, s0:s0 + (tc1 - tc0)],
                                          sp[:, s0:s0 + (tc1 - tc0)],
                                          Fm[:, ft0:ft0 + (tc1 - tc0)])
                        e_t = es[(j, kt, esb)]
                        nc.scalar.activation(
                            e_t[:, l0:l1], sp[:, s0:s0 + (tc1 - tc0)], EXP,
                            bias=eb[:, qc * 6 + kt:qc * 6 + kt + 1], scale=1.0)
                        # O^T += V_h (stationary) x es ; sums += ones x es
                        nc.tensor.matmul(
                            otp[j][p0:p0 + 64, ac0:ac1],
                            v_sb[4 * qc + kt][:, 64 * h:64 * h + 64],
                            e_t[:],
                            start=(kt == 0), stop=(kt == NKT - 1))
                        nc.tensor.matmul(
                            smp[j][p0:p0 + 64, ac0:ac1],
                            ones_b[:, 0:64],
                            e_t[:],
                            start=(kt == 0), stop=(kt == NKT - 1))
                oa = sb.tile([128, QCW], F16, tag=f"oa{pr}", bufs=2, name=f"oa{pr}")
                ss = sb.tile([128, QCW], F32, tag="ss", bufs=2, name="ss")
                rs = sb.tile([128, QCW], F32, tag="rs", bufs=2, name="rs")
                for j in range(2):
                    p0 = 64 * j
                    nc.any.tensor_copy(ss[p0:p0 + 64, :], smp[j][p0:p0 + 64, :])
                nc.vector.reciprocal(rs[:], ss[:])
                for j in range(2):
                    p0 = 64 * j
                    nc.any.tensor_mul(oa[p0:p0 + 64, :], otp[j][p0:p0 + 64, :],
                                      rs[p0:p0 + 64, :])
                oall[qc][pr] = oa

            # ---- phase D: output projection for this query chunk ----
            for rb in range(4):
                yp = pp.tile([128, C], F32, tag="gp", bufs=4, name="yp")
                for pr in range(4):
                    nc.tensor.matmul(
                        yp[:],
                        oall[qc][pr][:, 128 * rb:128 * (rb + 1)],
                        wo_sb[pr][:],
                        start=(pr == 0), stop=False)
                nc.tensor.matmul(yp[:], ones_b[0:1, :], bo[:],
                                 start=False, stop=True)
                ys = sb.tile([128, C], F32, tag="ys", bufs=3, name="ys")
                nc.any.tensor_copy(ys[:], yp[:])
                r0 = 512 * qc + 128 * rb
                nc.sync.dma_start(y[r0:r0 + 128, :], ys[:])


def build_nc():
    nc = bacc.Bacc("TRN2", target_bir_lowering=False, debug=False,
                   num_devices=NCORES)
    xh = nc.dram_tensor("xh", [ROWS, C], F16, kind="ExternalInput")
    wqkv = nc.dram_tensor("wqkv", [C, 3 * C], F16, kind="ExternalInput")
    wout = nc.dram_tensor("wout", [C, C], F16, kind="ExternalInput")
    bout = nc.dram_tensor("bout", [1, C], F16, kind="ExternalInput")
    ebias = nc.dram_tensor("ebias", [128, NQC * NKT], F32, kind="ExternalInput")
    y = nc.dram_tensor("y", [CHUNK, C], F32, kind="ExternalOutput")
    with tile.TileContext(nc) as tc:
        build_attention_body(tc, y[:], xh[:], wqkv[:], wout[:], bout[:], ebias[:])
    nc.compile()
    return nc


def make_in_maps(x, W_qkv, W_out, b_out):
    """Shard the full inputs into 8 per-core input maps."""
    x = np.asarray(x, dtype=np.float32)
    wqkv = np.asarray(W_qkv, dtype=np.float32).copy()
    wqkv[:, :C] *= SCALE  # fold hd^-0.5 into the q projection
    wqkv = wqkv.astype(np.float16)
    wout = np.asarray(W_out, dtype=np.float32).astype(np.float16)
    bo = np.asarray(b_out, dtype=np.float32).astype(np.float16).reshape(1, C)
    in_maps = []
    for core in range(NCORES):
        b, ch = divmod(core, 4)
        qs = CHUNK * ch
        xhalo = np.zeros((ROWS, C), dtype=np.float16)
        g0, g1 = qs - WIN, qs + CHUNK + WIN
        s0, s1 = max(g0, 0), min(g1, T)
        xhalo[s0 - g0:s1 - g0, :] = x[b, s0:s1, :].astype(np.float16)
        ebias = np.zeros((128, NQC * NKT), dtype=np.float32)
        for qc in range(NQC):
            for kt in range(NKT):
                k0 = qs + 512 * qc + 128 * kt - WIN  # global key-tile start
                if k0 < 0 or k0 >= T:
                    ebias[:, qc * 6 + kt] = NEG
        in_maps.append(dict(xh=xhalo, wqkv=wqkv, wout=wout, bout=bo, ebias=ebias))
    return in_maps


_CACHED_NC = None


def run_sharded(x, W_qkv, W_out, b_out, **run_kwargs):
    """Build (cached), run on 8 cores, gather. Returns (y_full, BassKernelResults)."""
    global _CACHED_NC
    if _CACHED_NC is None:
        _CACHED_NC = build_nc()
    in_maps = make_in_maps(x, W_qkv, W_out, b_out)
    res = run_bass_kernel_spmd(_CACHED_NC, in_maps, core_ids=list(range(NCORES)),
                               **run_kwargs)
    y_full = np.empty((B, T, C), dtype=np.float32)
    for core in range(NCORES):
        b, ch = divmod(core, 4)
        y_full[b, CHUNK * ch:CHUNK * (ch + 1), :] = res.results[core]["y"]
    return y_full, res


def kernel(x, W_qkv, W_out, b_out):
    y, _ = run_sharded(x, W_qkv, W_out, b_out)
    return y


# revision 20
# speedup vs baseline: 1.5505x; 1.1580x over previous
"""Banded (sliding-window) multi-head attention for TRN2, 8 NeuronCores.

Problem: nn_BaseAttention (B=2, T=4096, C=512, H=8, hd=64, WIN=128).
  qkv = x @ W_qkv ; banded softmax(q k^T / sqrt(hd), |i-j|<=WIN) @ v ; @ W_out + b_out

Sharding: 8 cores = 2 batches x 4 T-chunks of 1024 queries. Each core gets its
x rows plus a 128-row halo on each side (zero-padded at sequence edges) and
full replicated weights; it produces its own [1024, 512] output slice, so the
host-side gather is pure concatenation (no cross-core reduction).

Device pipeline per core (all layouts chosen to avoid transposing activations):
  xT   = x^T via PE transposes                      [C, 1280]
  q^T/k^T = W_qkv-slice^T-free matmuls (lhsT = W)   [hd, rows]  (head-major)
  v    = natural matmuls (lhsT = xT)                [rows, hd]
  S^T  = k^T-stationary matmuls -> PSUM             [keys, qcols]
  +band mask add (shifted-diagonal strip F) ; exp on ACT (edge tiles killed
  via per-partition bias) -> es in SBUF
  O^T  = sum_kt V-stationary matmuls over es        [2*hd, qcols]
  sums = ones-stationary matmuls over es (64-wide replicated rows)
  O^T * recip(sums) -> O_all^T ; Y = O_all^T-stationary @ W_out + b_out
"""

import os
import numpy as np

import concourse.bass as bass
from concourse import bacc
import concourse.mybir as mybir
import concourse.tile as tile
from concourse.bass_utils import run_bass_kernel_spmd
from concourse.masks import make_identity

# ----- problem constants (hardcoded per contest contract) -----
B, T, C = 2, 4096, 512
H, HD, WIN = 8, 64, 128
NCORES = 8
CHUNK = 1024                # queries per core
ROWS = CHUNK + 2 * WIN      # 1280 rows incl. halo
QCW = 512                   # query-chunk width (qcols per S^T tile group)
NQC = CHUNK // QCW          # 2
NKT = (QCW + 2 * WIN) // 128  # 6 key tiles per query-chunk
NEG = -30000.0
SCALE = HD ** -0.5

F32 = mybir.dt.float32
F32R = mybir.dt.float32r
BF16 = mybir.dt.bfloat16
F16 = mybir.dt.float16
EXP = mybir.ActivationFunctionType.Exp

# per-key-tile geometry: d = key-tile offset rel. to query-chunk start
# true subrect (cols that contain any in-band entry) and the widened matmul
# rect (>=256 wide so float32r matmuls run at full rate).
_KT_GEOM = []
for _kt in range(NKT):
    _d = 128 * _kt - 128
    _tc0 = max(0, _d - 128)
    _tc1 = min(QCW, _d + 256)
    _mc0, _mc1 = _tc0, _tc1
    if _mc1 - _mc0 < 256:  # widen (kt 0 and 5)
        if _mc0 == 0:
            _mc1 = 256
        else:
            _mc0 = QCW - 256
    _ft0 = _tc0 - _d + 128  # column offset into the band strip F
    # accumulating matmuls (O^T / sums): kt 0 opens the PSUM group and must
    # cover the full bank width so the sim's pending-zero state stays uniform
    # for the later partial-width accumulations.
    _ac0, _ac1 = (0, QCW) if _kt == 0 else (_mc0, _mc1)
    _KT_GEOM.append((_tc0, _tc1, _mc0, _mc1, _ft0, _ac0, _ac1))


def build_attention_body(tc, y, xh, wqkv, wout, bout, ebias):
    """Emit the per-core kernel. All APs are DRAM tensors.

    y     [1024, 512] out     xh    [1280, 512] in (halo'd x rows)
    wqkv  [512, 1536]  in (q-block pre-scaled by hd^-0.5 on host)
    wout  [512, 512]   in     bout  [1, 512] in
    ebias [128, 12]    in (col qc*6+kt: 0 or NEG for out-of-sequence key tiles)
    """
    nc = tc.nc
    from contextlib import ExitStack

    with ExitStack() as ctx:
        sb = ctx.enter_context(tc.tile_pool(name="sb", bufs=1))
        pp = ctx.enter_context(tc.tile_pool(name="pp", bufs=1, space="PSUM"))

        # ---- constants / persistent tiles ----
        ones_f = sb.tile([128, 128], F32, tag="ones_f", name="ones_f")
        nc.gpsimd.memset(ones_f[:], 1.0)
        ones_b = sb.tile([128, 128], F16, tag="ones_b", name="ones_b")
        nc.vector.tensor_copy(ones_b[:], ones_f[:])
        zeros_f = sb.tile([128, 384], F32, tag="zeros_f", name="zeros_f")
        nc.gpsimd.memset(zeros_f[:], 0.0)

        eb = sb.tile([128, NQC * NKT], F32, tag="eb", name="eb")
        nc.sync.dma_start(eb[:], ebias[:])
        bo = sb.tile([1, C], F16, tag="bo", name="bo")
        nc.sync.dma_start(bo[:], bout[:])
        wq_sb = []
        for i in range(4):
            w_i = sb.tile([128, 3 * C], F16, tag=f"wq{i}", name=f"wq{i}")
            nc.sync.dma_start(w_i[:], wqkv[128 * i:128 * (i + 1), :])
            wq_sb.append(w_i)
        wo_sb = []
        for i in range(4):
            w_i = sb.tile([128, C], F16, tag=f"wo{i}", name=f"wo{i}")
            nc.sync.dma_start(w_i[:], wout[128 * i:128 * (i + 1), :])
            wo_sb.append(w_i)

        xT = [sb.tile([128, ROWS], F16, tag=f"xT{i}", name=f"xT{i}") for i in range(4)]
        qT = [sb.tile([128, CHUNK], F16, tag=f"qT{i}", name=f"qT{i}") for i in range(4)]
        kT = [sb.tile([128, ROWS], F16, tag=f"kT{i}", name=f"kT{i}") for i in range(4)]
        v_sb = [sb.tile([128, C], F16, tag=f"v{i}", name=f"v{i}") for i in range(10)]

        # static exp-score tiles: es[j][kt][buf]  (j = head within pair)
        ESB = 2
        es = {}
        for j in range(2):
            for kt in range(NKT):
                tc0, tc1, _, _, _, ac0, ac1 = _KT_GEOM[kt]
                for bf in range(ESB):
                    t_e = sb.tile([128, ac1 - ac0], F16, tag=f"es{j}_{kt}_{bf}",
                                  name=f"es{j}_{kt}_{bf}")
                    es[(j, kt, bf)] = t_e
                    # zero the never-written complement once; those columns
                    # stay 0 forever (exp only ever writes the true subrect).
                    if tc0 - ac0 > 0:
                        nc.vector.tensor_copy(t_e[:, 0:tc0 - ac0],
                                              zeros_f[:, 0:tc0 - ac0])
                    if ac1 - tc1 > 0:
                        nc.vector.tensor_copy(t_e[:, tc1 - ac0:ac1 - ac0],
                                              zeros_f[:, 0:ac1 - tc1])

        # ---- phase A: xT = x^T via DMA transpose (2-byte xbar path) ----
        for ct in range(4):
            nc.sync.dma_start_transpose(xT[ct][:],
                                        xh[:, 128 * ct:128 * (ct + 1)])

        # ---- phase B: projections ----
        # q^T / k^T: out[feat, rows]; lhsT = W_qkv block, rhs = xT
        for ft in range(8):
            if ft < 4:  # q feats, own rows only (local rows [128, 1152))
                chunks = [(128, 512), (640, 512)]
                dest, doff = qT[ft], -128
            else:       # k feats, all rows
                chunks = [(0, 512), (512, 512), (1024, 256)]
                dest, doff = kT[ft - 4], 0
            for r0, rw in chunks:
                mm = pp.tile([128, QCW], F32, tag="gp", bufs=4, name="mmqk")
                for ct in range(4):
                    nc.tensor.matmul(
                        mm[:, 0:rw],
                        wq_sb[ct][:, 128 * ft:128 * (ft + 1)],
                        xT[ct][:, r0:r0 + rw],
                        start=(ct == 0), stop=(ct == 3))
                nc.vector.tensor_copy(dest[:, r0 + doff:r0 + doff + rw], mm[:, 0:rw])
        # v natural: out[rows, vfeat]; lhsT = xT tile, rhs = W_qkv v-block
        for rt in range(10):
            mm = pp.tile([128, QCW], F32, tag="gp", bufs=4, name="mmv")
            for ct in range(4):
                nc.tensor.matmul(
                    mm[:],
                    xT[ct][:, 128 * rt:128 * (rt + 1)],
                    wq_sb[ct][:, 1024:1536],
                    start=(ct == 0), stop=(ct == 3))
            nc.vector.tensor_copy(v_sb[rt][:], mm[:])

        # ---- phase C: banded attention ----
        # The sim only supports one PSUM accumulation group per bank region,
        # so each head-in-pair (j) gets its own O^T and sums banks, placed at
        # partition base 64*j so the elementwise normalize stays lane-aligned.
        oall = [[None] * 4 for _ in range(NQC)]
        for qc in range(NQC):
            for pr in range(4):
                otp = [pp.tile([128, QCW], F32, tag=f"av{j}", bufs=1,
                               name=f"otp{j}") for j in range(2)]
                smp = [pp.tile([128, QCW], F32, tag=f"sm{j}", bufs=1,
                               name=f"smp{j}") for j in range(2)]
                esb = (qc * 4 + pr) % ESB
                for kt in range(NKT):
                    tc0, tc1, mc0, mc1, ft0, ac0, ac1 = _KT_GEOM[kt]
                    w = mc1 - mc0
                    l0, l1 = tc0 - ac0, tc1 - ac0  # true region in es cols
                    kcol = 512 * qc + 128 * kt
                    for j in range(2):
                        h = 2 * pr + j
                        p0 = 64 * j
                        sp = pp.tile([128, 384], F32, tag="gp", bufs=4, name="sp")
                        # S^T = k^T (stationary) x q^T (moving)
                        nc.tensor.matmul(
                            sp[:, 0:w],
                            kT[h // 2][p0:p0 + 64, kcol:kcol + 128],
                            qT[h // 2][p0:p0 + 64,
                                             512 * qc + mc0:512 * qc + mc1],
                            start=True, stop=True)
                        # exp (edge tiles killed via bias), then zero the
                        # out-of-band entries with gpsimd affine selects
                        s0 = tc0 - mc0
                        e_t = es[(j, kt, esb)]
                        nc.scalar.activation(
                            e_t[:, l0:l1], sp[:, s0:s0 + (tc1 - tc0)], EXP,
                            bias=eb[:, qc * 6 + kt:qc * 6 + kt + 1], scale=1.0)
                        wt = tc1 - tc0
                        nc.gpsimd.affine_select(
                            out=e_t[:, l0:l1], in_=e_t[:, l0:l1],
                            compare_op=mybir.AluOpType.is_ge, fill=0.0,
                            base=ft0, pattern=[[1, wt]], channel_multiplier=-1)
                        nc.gpsimd.affine_select(
                            out=e_t[:, l0:l1], in_=e_t[:, l0:l1],
                            compare_op=mybir.AluOpType.is_ge, fill=0.0,
                            base=256 - ft0, pattern=[[-1, wt]],
                            channel_multiplier=1)
                        # O^T += V_h (stationary) x es ; sums += ones x es
                        nc.tensor.matmul(
                            otp[j][p0:p0 + 64, ac0:ac1],
                            v_sb[4 * qc + kt][:, 64 * h:64 * h + 64],
                            e_t[:],
                            start=(kt == 0), stop=(kt == NKT - 1))
                        nc.tensor.matmul(
                            smp[j][p0:p0 + 64, ac0:ac1],
                            ones_b[:, 0:64],
                            e_t[:],
                            start=(kt == 0), stop=(kt == NKT - 1))
                oa = sb.tile([128, QCW], F16, tag=f"oa{pr}", bufs=2, name=f"oa{pr}")
                ss = sb.tile([128, QCW], F32, tag="ss", bufs=2, name="ss")
                rs = sb.tile([128, QCW], F32, tag="rs", bufs=2, name="rs")
                for j in range(2):
                    p0 = 64 * j
                    nc.any.tensor_copy(ss[p0:p0 + 64, :], smp[j][p0:p0 + 64, :])
                nc.vector.reciprocal(rs[:], ss[:])
                for j in range(2):
                    p0 = 64 * j
                    nc.any.tensor_mul(oa[p0:p0 + 64, :], otp[j][p0:p0 + 64, :],
                                      rs[p0:p0 + 64, :])
                oall[qc][pr] = oa

            # ---- phase D: output projection for this query chunk ----
            for rb in range(4):
                yp = pp.tile([128, C], F32, tag="gp", bufs=4, name="yp")
                for pr in range(4):
                    nc.tensor.matmul(
                        yp[:],
                        oall[qc][pr][:, 128 * rb:128 * (rb + 1)],
                        wo_sb[pr][:],
                        start=(pr == 0), stop=False)
                nc.tensor.matmul(yp[:], ones_b[0:1, :], bo[:],
                                 start=False, stop=True)
                ys = sb.tile([128, C], F32, tag="ys", bufs=3, name="ys")
                nc.any.tensor_copy(ys[:], yp[:])
                r0 = 512 * qc + 128 * rb
                nc.sync.dma_start(y[r0:r0 + 128, :], ys[:])


def build_nc():
    nc = bacc.Bacc("TRN2", target_bir_lowering=False, debug=False,
                   num_devices=NCORES)
    xh = nc.dram_tensor("xh", [ROWS, C], F16, kind="ExternalInput")
    wqkv = nc.dram_tensor("wqkv", [C, 3 * C], F16, kind="ExternalInput")
    wout = nc.dram_tensor("wout", [C, C], F16, kind="ExternalInput")
    bout = nc.dram_tensor("bout", [1, C], F16, kind="ExternalInput")
    ebias = nc.dram_tensor("ebias", [128, NQC * NKT], F32, kind="ExternalInput")
    y = nc.dram_tensor("y", [CHUNK, C], F32, kind="ExternalOutput")
    with tile.TileContext(nc) as tc:
        build_attention_body(tc, y[:], xh[:], wqkv[:], wout[:], bout[:], ebias[:])
    nc.compile()
    return nc


def make_in_maps(x, W_qkv, W_out, b_out):
    """Shard the full inputs into 8 per-core input maps."""
    x = np.asarray(x, dtype=np.float32)
    wqkv = np.asarray(W_qkv, dtype=np.float32).copy()
    wqkv[:, :C] *= SCALE  # fold hd^-0.5 into the q projection
    wqkv = wqkv.astype(np.float16)
    wout = np.asarray(W_out, dtype=np.float32).astype(np.float16)
    bo = np.asarray(b_out, dtype=np.float32).astype(np.float16).reshape(1, C)
    in_maps = []
    for core in range(NCORES):
        b, ch = divmod(core, 4)
        qs = CHUNK * ch
        xhalo = np.zeros((ROWS, C), dtype=np.float16)
        g0, g1 = qs - WIN, qs + CHUNK + WIN
        s0, s1 = max(g0, 0), min(g1, T)
        xhalo[s0 - g0:s1 - g0, :] = x[b, s0:s1, :].astype(np.float16)
        ebias = np.zeros((128, NQC * NKT), dtype=np.float32)
        for qc in range(NQC):
            for kt in range(NKT):
                k0 = qs + 512 * qc + 128 * kt - WIN  # global key-tile start
                if k0 < 0 or k0 >= T:
                    ebias[:, qc * 6 + kt] = NEG
        in_maps.append(dict(xh=xhalo, wqkv=wqkv, wout=wout, bout=bo, ebias=ebias))
    return in_maps


_CACHED_NC = None


def run_sharded(x, W_qkv, W_out, b_out, **run_kwargs):
    """Build (cached), run on 8 cores, gather. Returns (y_full, BassKernelResults)."""
    global _CACHED_NC
    if _CACHED_NC is None:
        _CACHED_NC = build_nc()
    in_maps = make_in_maps(x, W_qkv, W_out, b_out)
    res = run_bass_kernel_spmd(_CACHED_NC, in_maps, core_ids=list(range(NCORES)),
                               **run_kwargs)
    y_full = np.empty((B, T, C), dtype=np.float32)
    for core in range(NCORES):
        b, ch = divmod(core, 4)
        y_full[b, CHUNK * ch:CHUNK * (ch + 1), :] = res.results[core]["y"]
    return y_full, res


def kernel(x, W_qkv, W_out, b_out):
    y, _ = run_sharded(x, W_qkv, W_out, b_out)
    return y
